# revision 21
# baseline (speedup 1.0000x reference)
"""BiLSTM-CRF Trainium2 kernel (8-core data-parallel over batch).

Per core: 8 examples. Design:
  0) masked-token embedding gather (fp32 table + zero row) -> PE transpose ->
     segment-replicated x^T layout (fp32r), bias/mask folded in as extra row
  1) (fused into 2) JIT fp32r input-projection GEMM per recurrence chunk
  2) segmented LSTM recurrence (S=4 segments, W=32 warmup -> 160 wall steps),
     fwd+bwd interleaved; bf16 h feedback, fp32 cell state
  3) feats GEMM (plain fp32) + bias; partition remap for viterbi
  4) forward AND backward Viterbi max-plus scans (tensor_scalar + transposed
     reduce_max, streamed mask-selected transitions from host)
  5) max-marginal decode: argmax_j(fwd[t]+bwd[t]+feat[t]) per position in bulk
     (no backpointers, no backtrace)
"""
import os
import sys

sys.path.insert(0, "/opt/trn_rl_repo")

import numpy as np
import ml_dtypes
from contextlib import ExitStack

B, T, V, E = 64, 512, 100000, 300
H = 256
G4 = 4 * H
NB = 8                 # examples per core
L = 20
L2 = 22
START, STOP = 20, 21
NCORES = 8
NEG = -100.0           # identity-transition off-diagonal
PAD = -200.0           # padded rows/cols in viterbi tiles

SEG = 4                # recurrence segments
TS = T // SEG          # 128
W = 32                 # warmup steps
NSTEP = TS + W         # 160 wall steps per direction
NI = NB * SEG          # 32 chain instances (col order: e*SEG + s)
CH = 8                 # JIT pregate chunk (steps)
NCH = NSTEP // CH      # 20
IB = TS + 2 * W        # 192 i-slots per (e,s) block in xT
XC = NB * SEG * IB     # 6144 xT columns; col = (e*SEG+s)*IB + i; t = s*TS+i-W
KCH = [(0, 128, 128), (128, 256, 128), (256, 301, 45)]  # (row0, row1, kw); 300=mask/bias row

_CACHE = {}


def _build_program(debug=False):
    import concourse.bass as bass
    import concourse.tile as tile
    from concourse import bacc, mybir

    f32 = mybir.dt.float32
    f32r = mybir.dt.float32r
    bf16 = mybir.dt.bfloat16
    i32 = mybir.dt.int32
    AF = mybir.ActivationFunctionType
    OP = mybir.AluOpType
    AX = mybir.AxisListType

    nc = bacc.Bacc("TRN2", target_bir_lowering=False, debug=False,
                   enable_asserts=False, num_devices=NCORES)

    def inp(name, shape, dtype):
        return nc.dram_tensor(name, shape, dtype, kind="ExternalInput").ap()

    WORDS = inp("words", [128, 32], i32)            # masked ids; tok = c*128+p
    ETAB = inp("etab", [V + 1, E], f32r)            # + zero row V
    WIH = {d: inp(f"wihT_{d}", [384, G4], f32r) for d in "fb"}
    WHH = {d: inp(f"whhT_{d}", [H, G4], bf16) for d in "fb"}
    MASKXT = inp("maskxt", [1, XC], f32r)
    IDENT = inp("ident", [128, 128], f32r)
    WOUT = inp("woutT", [2 * H, L2], f32)
    BOUT = inp("bout", [L2, 1], f32)
    TSELF = [inp(f"tself{g}", [128, T * 32], f32) for g in range(2)]
    TSELB = [inp(f"tselb{g}", [128, T * 32], f32) for g in range(2)]
    MASKV = [inp(f"maskv{g}", [128, T], f32) for g in range(2)]
    TSTART = inp("tstart_col", [128, 1], f32)
    TSTOP = inp("tstop_col", [128, 1], f32)
    IOTAREV = inp("iotarev512", [128, T], f32)
    MASKSCR = [inp(f"maskscr{g}", [128, 16], f32) for g in range(2)]

    # scrambled tag layout: pathout[g, 32e+i, u] = tag(example 4g+e, t=32u+i)
    PATH_OUT = nc.dram_tensor("pathout", [2, 128, 16], i32,
                              kind="ExternalOutput").ap()
    if debug:
        DBG_H = {d: nc.dram_tensor(f"dbg_h_{d}", [128, 2, NB, T], f32,
                                   kind="ExternalOutput").ap() for d in "fb"}
        DBG_FV = [nc.dram_tensor(f"dbg_fv{g}", [128, T], f32,
                                 kind="ExternalOutput").ap() for g in range(2)]
        DBG_PF = [nc.dram_tensor(f"dbg_pf{g}", [128, T], f32,
                                 kind="ExternalOutput").ap() for g in range(2)]
        DBG_PB = [nc.dram_tensor(f"dbg_pb{g}", [128, T], f32,
                                 kind="ExternalOutput").ap() for g in range(2)]

    with tile.TileContext(nc) as tc:
        with ExitStack() as ctx:
            cst = ctx.enter_context(tc.tile_pool(name="cst", bufs=1))

            # ---- constants ----
            whh_sb = {d: cst.tile([128, 2, G4], bf16, tag=f"whh{d}", name=f"whh{d}")
                      for d in "fb"}
            for d in "fb":
                nc.sync.dma_start(whh_sb[d][:, 0, :], WHH[d][0:128, :])
                nc.sync.dma_start(whh_sb[d][:, 1, :], WHH[d][128:256, :])
            wih_sb = {d: cst.tile([128, 3, G4], f32r, tag=f"wih{d}", name=f"wih{d}")
                      for d in "fb"}
            for d in "fb":
                for k in range(3):
                    r0, r1, kw = KCH[k]
                    nc.sync.dma_start(wih_sb[d][:kw, k, :],
                                      WIH[d][k * 128:k * 128 + kw, :])
            if True:
                h_out = {d: cst.tile([128, 2, NB, T], f32, tag=f"ho{d}",
                                     name=f"ho{d}") for d in "fb"}

                # ================= phase 0: gather + transpose =================
                with tc.tile_pool(name="xtp", bufs=1) as xtp:
                    xt = [xtp.tile([128, XC], f32r, tag=f"xt{k}", name=f"xt{k}")
                          for k in range(3)]
                    # zero pads: (s=0, i<W) and (s=SEG-1, i>=IB-W) per (e)
                    for k in range(3):
                        kw = KCH[k][2]
                        kwz = kw if k < 2 else kw - 1     # mask row written below
                        z0 = xt[k][:kwz, :].bitcast(mybir.dt.uint32).rearrange(
                            "p (q i) -> p q i", i=IB)
                        nc.vector.memset(z0[:, 0::SEG, 0:W], 0)
                        nc.vector.memset(z0[:, SEG - 1::SEG, IB - W:IB], 0)
                    nc.sync.dma_start(xt[2][44:45, :], MASKXT[:])

                    idx_all = cst.tile([128, 32], i32, tag="idx", name="idx")
                    nc.sync.dma_start(idx_all[:], WORDS[:])
                    with tc.tile_pool(name="ph0", bufs=4) as p0, \
                         tc.tile_pool(name="ph0ps", bufs=4, space="PSUM") as p0ps:
                        ident = p0.tile([128, 128], f32r, tag="ident",
                                        name="ident")
                        nc.sync.dma_start(ident[:], IDENT[:])
                        for c in range(32):
                            e, sseg = divmod(c, SEG)
                            base = (e * SEG + sseg) * IB + W
                            idxc = p0.tile([128, 1], i32, tag="idxc", name="idxc")
                            nc.vector.tensor_copy(idxc[:], idx_all[:, c:c + 1])
                            femb = p0.tile([128, E], f32r, tag="femb", name="femb")
                            nc.gpsimd.indirect_dma_start(
                                out=femb[:], out_offset=None, in_=ETAB[:],
                                in_offset=bass.IndirectOffsetOnAxis(
                                    ap=idxc[:, :1], axis=0))
                            for k in range(3):
                                r0, r1, kw = KCH[k]
                                kwe = min(r1, E) - r0          # 128/128/44 rows
                                pst = p0ps.tile([128, 128], f32r, tag="tps",
                                                name="tps")
                                nc.tensor.transpose(pst[:kwe, :],
                                                    femb[:, r0:r0 + kwe],
                                                    ident[:])
                                if (c + k) % 2:
                                    nc.scalar.copy(
                                        xt[k][:kwe, base:base + 128], pst[:kwe, :])
                                else:
                                    nc.vector.tensor_copy(
                                        xt[k][:kwe, base:base + 128], pst[:kwe, :])
                                # warmup duplicates
                                if sseg < SEG - 1:   # tail 32 -> next block head
                                    nc.vector.tensor_copy(
                                        xt[k][:kwe,
                                              (e * SEG + sseg + 1) * IB:
                                              (e * SEG + sseg + 1) * IB + W],
                                        pst[:kwe, 128 - W:128])
                                if sseg > 0:         # head 32 -> prev block tail
                                    nc.scalar.copy(
                                        xt[k][:kwe,
                                              (e * SEG + sseg - 1) * IB + IB - W:
                                              (e * SEG + sseg) * IB],
                                        pst[:kwe, 0:W])

                    # ============ phase 2: fused JIT projection + recurrence ====
                    with tc.tile_pool(name="pgps", bufs=4, space="PSUM") as pgps, \
                         tc.tile_pool(name="recps", bufs=2, space="PSUM") as rps, \
                         tc.tile_pool(name="pgstg", bufs=2) as stgp, \
                         tc.tile_pool(name="chain", bufs=1) as chp, \
                         tc.tile_pool(name="state", bufs=1) as stp:
                        c_t = {d: stp.tile([128, 2, NI], f32, tag=f"c{d}",
                                           name=f"c{d}") for d in "fb"}
                        hbf = {d: stp.tile([128, 2, NI], bf16, tag=f"hb{d}",
                                           name=f"hb{d}") for d in "fb"}
                        for d in "fb":
                            nc.vector.memset(c_t[d][:], 0)
                            nc.vector.memset(hbf[d][:], 0)

                        pg_stage = {d: [None] * NCH for d in "fb"}

                        def issue_gemm_unit(kchunk, d, m):
                            """JIT pregates for (dir d, gate block m) of chunk."""
                            i0 = kchunk * CH
                            if d == "f":
                                sl = (i0, i0 + CH)
                            else:
                                sl = (IB - i0 - CH, IB - i0)
                            if pg_stage[d][kchunk] is None:
                                pg_stage[d][kchunk] = stgp.tile(
                                    [128, 8, 8 * SEG * CH], f32,
                                    tag=f"pgs{d}", name=f"pgs{d}")
                            ps = pgps.tile([128, 8 * SEG * CH], f32,
                                           tag="pgm", name="pgm")
                            for k in range(3):
                                kw = KCH[k][2]
                                rhs = xt[k][:kw, :].rearrange(
                                    "p (q i) -> p q i", i=IB)[:, :, sl[0]:sl[1]]
                                nc.tensor.matmul(
                                    ps[:], lhsT=wih_sb[d][:kw, k,
                                                          m * 128:(m + 1) * 128],
                                    rhs=rhs, start=(k == 0), stop=(k == 2))
                            nc.scalar.copy(pg_stage[d][kchunk][:, m, :], ps[:])

                        def step(d, i):
                            kchunk, ii = divmod(i, CH)
                            # pregate slot for this step: [128, 8m, 8e, SEG]
                            stg = pg_stage[d][kchunk][:, :, :].rearrange(
                                "p m (q c) -> p m q c", c=CH)
                            cc = ii if d == "f" else CH - 1 - ii
                            pg = stg[:, :, :, cc]            # [128, 8, 32]
                            ps = rps.tile([128, 8, NI], f32, tag=f"g{d}",
                                          name=f"g{d}")
                            for m in range(8):
                                for kk in range(2):
                                    nc.tensor.matmul(
                                        ps[:, m, :],
                                        lhsT=whh_sb[d][:, kk,
                                                       m * 128:(m + 1) * 128],
                                        rhs=hbf[d][:, kk, :],
                                        start=(kk == 0), stop=(kk == 1))
                            gs = chp.tile([128, 8, NI], f32, tag=f"gs{d}",
                                          name=f"gs{d}")
                            nc.vector.tensor_add(gs[:], ps[:], pg)
                            # gate order (i, f, o, g)
                            s_ifo = chp.tile([128, 6, NI], f32, tag=f"si{d}",
                                             name=f"si{d}")
                            nc.scalar.activation(s_ifo[:], gs[:, 0:6, :],
                                                 AF.Sigmoid)
                            t_g = chp.tile([128, 2, NI], f32, tag=f"tg{d}",
                                           name=f"tg{d}")
                            nc.scalar.activation(t_g[:], gs[:, 6:8, :], AF.Tanh)
                            tmp = chp.tile([128, 2, NI], f32, tag=f"tm{d}",
                                           name=f"tm{d}")
                            nc.vector.tensor_mul(tmp[:], s_ifo[:, 0:2, :], t_g[:])
                            nc.vector.tensor_mul(c_t[d][:], c_t[d][:],
                                                 s_ifo[:, 2:4, :])
                            nc.vector.tensor_add(c_t[d][:], c_t[d][:], tmp[:])
                            t_c = chp.tile([128, 2, NI], f32, tag=f"tc{d}",
                                           name=f"tc{d}")
                            nc.scalar.activation(t_c[:], c_t[d][:], AF.Tanh)
                            nc.vector.tensor_mul(hbf[d][:], t_c[:],
                                                 s_ifo[:, 4:6, :])
                            if i >= W:
                                r = i - W if d == "f" else TS - 1 - (i - W)
                                dst = h_out[d][:, :, :, :].rearrange(
                                    "p k e (s r) -> p k e s r", s=SEG)[:, :, :, :, r]
                                nc.gpsimd.tensor_tensor(
                                    dst, t_c[:].rearrange("p k (e s) -> p k e s",
                                                          s=SEG),
                                    s_ifo[:, 4:6, :].rearrange(
                                        "p k (e s) -> p k e s", s=SEG),
                                    op=OP.mult)

                        for d in "fb":
                            for m in range(8):
                                issue_gemm_unit(0, d, m)
                        for kchunk in range(NCH):
                            for ii in range(CH):
                                i = kchunk * CH + ii
                                if kchunk + 1 < NCH and ii < 8:
                                    u = 2 * ii
                                    for uu in (u, u + 1):
                                        d2, m2 = ("f", uu) if uu < 8 else \
                                                 ("b", uu - 8)
                                        issue_gemm_unit(kchunk + 1, d2, m2)
                                step("f", i)
                                step("b", i)

                # late constants (opened after xt frees its SBUF)
                late = ctx.enter_context(tc.tile_pool(name="late", bufs=1))
                maskv = [late.tile([128, T], f32, tag=f"mv{g}", name=f"mv{g}")
                         for g in range(2)]
                iotarev = late.tile([128, T], f32, tag="iota", name="iota")
                tstart = late.tile([128, 1], f32, tag="tstart", name="tstart")
                tstop = late.tile([128, 1], f32, tag="tstop", name="tstop")
                maskscr = [late.tile([128, 16], f32, tag=f"ms{g}", name=f"ms{g}")
                           for g in range(2)]
                for apdst, apsrc in ((maskv[0], MASKV[0]), (maskv[1], MASKV[1]),
                                     (iotarev, IOTAREV), (tstart, TSTART),
                                     (tstop, TSTOP), (maskscr[0], MASKSCR[0]),
                                     (maskscr[1], MASKSCR[1])):
                    nc.sync.dma_start(apdst[:], apsrc[:])
                featsvit = [late.tile([128, T], f32, tag=f"fv{g}", name=f"fv{g}")
                            for g in range(2)]

                # ================= phase 3: feats =================
                if debug:
                    for d in "fb":
                        nc.sync.dma_start(DBG_H[d][:], h_out[d][:])
                with tc.tile_pool(name="fw", bufs=1) as fwp, \
                     tc.tile_pool(name="fps", bufs=4, space="PSUM") as fps:
                    wout = fwp.tile([128, 4, L2], f32, tag="wo", name="wo")
                    for k in range(4):
                        nc.sync.dma_start(wout[:, k, :],
                                          WOUT[k * 128:(k + 1) * 128, :])
                    bout_sb = fwp.tile([L2, 1], f32, tag="bo", name="bo")
                    nc.sync.dma_start(bout_sb[:], BOUT[:])
                    feats_sb = fwp.tile([L2, NB * T], f32, tag="fs", name="fs")
                    hsrc = [h_out["f"][:, 0, :, :], h_out["f"][:, 1, :, :],
                            h_out["b"][:, 0, :, :], h_out["b"][:, 1, :, :]]
                    for n in range(8):
                        ps = fps.tile([L2, 512], f32, tag="fp", name="fp")
                        for k in range(4):
                            flat = hsrc[k].rearrange("p e t -> p (e t)")
                            nc.tensor.matmul(
                                ps[:], lhsT=wout[:, k, :],
                                rhs=flat[:, n * 512:(n + 1) * 512],
                                start=(k == 0), stop=(k == 3))
                        nc.scalar.activation(feats_sb[:, n * 512:(n + 1) * 512],
                                             ps[:], AF.Identity,
                                             bias=bout_sb[:, 0:1])
                    for g in range(2):
                        nc.vector.memset(featsvit[g][:], 0)
                        for e in range(4):
                            b = 4 * g + e
                            nc.sync.dma_start(
                                featsvit[g][32 * e:32 * e + L2, :],
                                feats_sb[:, b * T:(b + 1) * T])
                        nc.vector.tensor_mul(featsvit[g][:], featsvit[g][:],
                                             maskv[g][:])

            # ================= phase 4: dual viterbi scans =================
            if debug:
                for g in range(2):
                    nc.sync.dma_start(DBG_FV[g][:], featsvit[g][:])
            pa_f = [late.tile([128, T], f32, tag=f"paf{g}", name=f"paf{g}")
                    for g in range(2)]
            pa_b = [late.tile([128, T], f32, tag=f"pab{g}", name=f"pab{g}")
                    for g in range(2)]
            CHV = 16
            with tc.tile_pool(name="vstr", bufs=2) as vsp:
                for g in range(2):
                    nc.vector.tensor_copy(pa_f[g][:, 0:1], tstart[:])
                    nc.vector.tensor_copy(pa_b[g][:, T - 1:T], tstop[:])
                strm = {}
                NQC = T // CHV

                def load_chunk(kind, g, qc):
                    src = (TSELF if kind == "f" else TSELB)[g]
                    tl = vsp.tile([128, CHV * 32], f32, tag=f"st{kind}{g}",
                                  name=f"st{kind}{g}")
                    nc.sync.dma_start(
                        tl[:], src[:, qc * CHV * 32:(qc + 1) * CHV * 32])
                    strm[(kind, g, qc % 2)] = tl

                for g in range(2):
                    for kind in "fb":
                        load_chunk(kind, g, 0)
                        load_chunk(kind, g, 1)
                for q in range(1, T):
                    tau = T - 1 - q
                    for g in range(2):
                        qc = q // CHV
                        if q % CHV == 0 and qc + 1 < NQC:
                            load_chunk("f", g, qc + 1)
                            load_chunk("b", g, qc + 1)
                        qq = q % CHV
                        slot_f = strm[("f", g, qc % 2)][:, qq * 32:(qq + 1) * 32]
                        cur = vsp.tile([128, 32], f32, tag=f"cf{g}",
                                       name=f"cf{g}")
                        nc.vector.tensor_scalar(
                            cur[:], slot_f, pa_f[g][:, q - 1:q],
                            featsvit[g][:, q - 1:q], op0=OP.add, op1=OP.add)
                        nc.vector.tensor_reduce(
                            pa_f[g][:, q:q + 1], cur[:], axis=AX.X, op=OP.max,
                            apply_transpose=True)
                        slot_b = strm[("b", g, qc % 2)][:, qq * 32:(qq + 1) * 32]
                        curb = vsp.tile([128, 32], f32, tag=f"cb{g}",
                                        name=f"cb{g}")
                        nc.vector.tensor_scalar(
                            curb[:], slot_b, pa_b[g][:, tau + 1:tau + 2],
                            featsvit[g][:, tau + 1:tau + 2],
                            op0=OP.add, op1=OP.add)
                        nc.vector.tensor_reduce(
                            pa_b[g][:, tau:tau + 1], curb[:], axis=AX.X,
                            op=OP.max, apply_transpose=True)

            # ================= phase 5: bulk max-marginal decode ============
            if debug:
                for g in range(2):
                    nc.sync.dma_start(DBG_PF[g][:], pa_f[g][:])
                    nc.sync.dma_start(DBG_PB[g][:], pa_b[g][:])
            with tc.tile_pool(name="dec", bufs=1) as dp:
                for g in range(2):
                    s_all = dp.tile([128, T], f32, tag=f"sa{g}", name=f"sa{g}")
                    nc.vector.tensor_add(s_all[:], pa_f[g][:], pa_b[g][:])
                    nc.vector.tensor_add(s_all[:], s_all[:], featsvit[g][:])
                    st = dp.tile([128, T], f32, tag=f"stt{g}", name=f"stt{g}")
                    nc.vector.transpose(st[:], s_all[:])
                    stv = st[:].rearrange("p (u v) -> p u v", v=32)
                    maxv = dp.tile([128, 16], f32, tag=f"mx{g}", name=f"mx{g}")
                    nc.vector.tensor_reduce(maxv[:], stv, axis=AX.X, op=OP.max)
                    eq = dp.tile([128, T], f32, tag=f"eq{g}", name=f"eq{g}")
                    mvb = maxv[:].rearrange("p u -> p u ()").broadcast_to(
                        (128, 16, 32))
                    nc.vector.tensor_tensor(
                        eq[:].rearrange("p (u v) -> p u v", v=32), stv, mvb,
                        op=OP.is_equal)
                    nc.vector.tensor_mul(eq[:], eq[:], iotarev[:])
                    brv = dp.tile([128, 16], f32, tag=f"br{g}", name=f"br{g}")
                    nc.vector.tensor_reduce(
                        brv[:], eq[:].rearrange("p (u v) -> p u v", v=32),
                        axis=AX.X, op=OP.max)
                    nc.vector.tensor_scalar(brv[:], brv[:], -1.0, 31.0,
                                            op0=OP.mult, op1=OP.add)
                    nc.vector.tensor_mul(brv[:], brv[:], maskscr[g][:])
                    tagi = dp.tile([128, 16], i32, tag=f"ti{g}", name=f"ti{g}")
                    nc.vector.tensor_copy(tagi[:], brv[:])
                    nc.sync.dma_start(PATH_OUT[g, :, :], tagi[:])

    nc.compile()
    return nc


def _prep_inputs(inputs):
    f32 = np.float32
    bf = ml_dtypes.bfloat16

    mask = np.ascontiguousarray(np.asarray(inputs["mask"])).astype(f32)
    words = np.ascontiguousarray(np.asarray(inputs["batch_word"])).astype(np.int32)
    words_m = np.where(mask > 0, words, V).astype(np.int32)

    etab = np.zeros((V + 1, E), f32)
    etab[:V] = np.asarray(inputs["emb_table"], f32)
    trans = np.asarray(inputs["trans"], f32)

    # gate order permutation (i, f, g, o) -> (i, f, o, g); blocks of 128 cols
    blk = [0, 1, 2, 3, 6, 7, 4, 5]

    def permute_cols(w):            # w: [4H, x] rows are gate blocks
        return np.concatenate([w[128 * b:128 * (b + 1)] for b in blk], axis=0)

    shared = {"etab": etab, "ident": np.eye(128, dtype=f32)}
    for d, (wih, whh, bih, bhh) in (
            ("f", ("Wih_f", "Whh_f", "bih_f", "bhh_f")),
            ("b", ("Wih_b", "Whh_b", "bih_b", "bhh_b"))):
        Wih = permute_cols(np.asarray(inputs[wih], f32))       # [4H, E]
        Whh = permute_cols(np.asarray(inputs[whh], f32))       # [4H, H]
        bias = permute_cols((np.asarray(inputs[bih], f32)
                             + np.asarray(inputs[bhh], f32))[:, None])[:, 0]
        wt = np.zeros((384, G4), f32)
        wt[0:128] = Wih.T[0:128]
        wt[128:256] = Wih.T[128:256]
        wt[256:300] = Wih.T[256:300]
        wt[300] = bias
        shared[f"wihT_{d}"] = wt
        shared[f"whhT_{d}"] = np.ascontiguousarray(Whh.T).astype(bf)  # [H, 4H]

    shared["woutT"] = np.ascontiguousarray(
        np.asarray(inputs["W_out"], f32).T)                    # [2H, L2]
    shared["bout"] = np.ascontiguousarray(
        np.asarray(inputs["b_out"], f32).reshape(L2, 1))

    id_m = np.where(np.eye(L2, dtype=bool), 0.0, NEG).astype(f32)
    tr22 = trans[:L2, :L2]
    # 32x32 padded variants; pads PAD
    trans32 = np.full((32, 32), PAD, f32); trans32[:L2, :L2] = tr22
    id32 = np.full((32, 32), PAD, f32); id32[:L2, :L2] = id_m

    tstart_col = np.full((128, 1), PAD, f32)
    tstop_col = np.full((128, 1), PAD, f32)
    for e in range(4):
        tstart_col[32 * e:32 * e + L2, 0] = trans[START, :L2]
        tstop_col[32 * e:32 * e + L2, 0] = trans[:L2, STOP]
    iotarev512 = np.zeros((128, T), f32)
    for u in range(16):
        iotarev512[:, 32 * u:32 * u + 32] = (31.0 - np.arange(32, dtype=f32))[None, :]
    shared["tstart_col"] = tstart_col
    shared["tstop_col"] = tstop_col
    shared["iotarev512"] = iotarev512

    in_maps = []
    for c in range(NCORES):
        m = dict(shared)
        bsl = slice(c * NB, (c + 1) * NB)
        wl = words_m[bsl].reshape(NB * T)
        m["words"] = np.ascontiguousarray(wl.reshape(32, 128).T)
        mk = mask[bsl]                                        # [8, T]
        # maskxt row: col (e*SEG+s)*IB + i -> t = s*TS + i - W
        mrow = np.zeros((1, XC), f32)
        for e in range(NB):
            for s in range(SEG):
                base = (e * SEG + s) * IB
                t0 = s * TS - W
                for i in range(IB):
                    t = t0 + i
                    if 0 <= t < T:
                        mrow[0, base + i] = mk[e, t]
        m["maskxt"] = mrow
        for g in range(2):
            mv = np.zeros((128, T), f32)
            msc = np.zeros((128, 16), f32)
            tsf = np.full((128, T * 32), PAD, f32)
            tsb = np.full((128, T * 32), PAD, f32)
            for e in range(4):
                ex = 4 * g + e
                mv[32 * e:32 * e + L2, :] = mk[ex][None, :]
                for i in range(32):
                    msc[32 * e + i, :] = mk[ex, i::32]
                # fwd slots q=1..T-1: tsel[q] = m[q] ? trans : id ([j, i])
                me = mk[ex]                                   # [T]
                sel = np.where(me[:, None, None] > 0, trans32[None],
                               id32[None])                    # [T, 32, 32]
                tsf[32 * e:32 * e + 32, :] = np.ascontiguousarray(
                    sel.transpose(1, 0, 2).reshape(32, T * 32))
                # bwd slot q (q=1..T-1): tau = T-1-q uses tsel[tau+1] transposed
                selb = np.where(me[None, :, None] > 0, trans32.T[:, None, :],
                                id32[:, None, :])             # [32, T, 32] (i, t, j)
                # slot q content cols q*32+j = sel at t=tau+1=T-q
                ordb = selb[:, (T - np.arange(T)) % T, :]     # q=0 unused
                tsb[32 * e:32 * e + 32, :] = np.ascontiguousarray(
                    ordb.reshape(32, T * 32))
            m[f"maskv{g}"] = mv
            m[f"maskscr{g}"] = msc
            m[f"tself{g}"] = tsf
            m[f"tselb{g}"] = tsb
        in_maps.append(m)
    return in_maps


def kernel(**inputs):
    from concourse.bass_utils import run_bass_kernel_spmd

    debug = bool(int(os.environ.get("KDEBUG", "0")))
    key = ("nc", debug)
    if key not in _CACHE:
        _CACHE[key] = _build_program(debug)
    nc = _CACHE[key]

    in_maps = _prep_inputs(inputs)
    res = run_bass_kernel_spmd(nc, in_maps, core_ids=list(range(NCORES)))
    _CACHE["last_results"] = res
    out = np.zeros((B, T), np.int32)
    for c in range(NCORES):
        scr = res.results[c]["pathout"]          # [2, 128, 16]
        for g in range(2):
            blk = scr[g].reshape(4, 32, 16)      # [e, i, u]
            out[c * NB + 4 * g:c * NB + 4 * g + 4] = (
                blk.transpose(0, 2, 1).reshape(4, T))
    return out.astype(np.int32)


# revision 27
# speedup vs baseline: 1.0484x; 1.0484x over previous
"""BiLSTM-CRF Trainium2 kernel (8-core data-parallel over batch).

Per core: 8 examples. Design:
  0) masked-token embedding gather (fp32 table + zero row) -> PE transpose ->
     segment-replicated x^T layout (fp32r), bias/mask folded in as extra row
  1) (fused into 2) JIT fp32r input-projection GEMM per recurrence chunk
  2) segmented LSTM recurrence (S=4 segments, W=32 warmup -> 160 wall steps),
     fwd+bwd interleaved; bf16 h feedback, fp32 cell state
  3) feats GEMM (plain fp32) + bias; partition remap for viterbi
  4) forward AND backward Viterbi max-plus scans (tensor_scalar + transposed
     reduce_max, streamed mask-selected transitions from host)
  5) max-marginal decode: argmax_j(fwd[t]+bwd[t]+feat[t]) per position in bulk
     (no backpointers, no backtrace)
"""
import os
import sys

sys.path.insert(0, "/opt/trn_rl_repo")

import numpy as np
import ml_dtypes
from contextlib import ExitStack

B, T, V, E = 64, 512, 100000, 300
H = 256
G4 = 4 * H
NB = 8                 # examples per core
L = 20
L2 = 22
START, STOP = 20, 21
NCORES = 8
NEG = -100.0           # identity-transition off-diagonal
PAD = -200.0           # padded rows/cols in viterbi tiles

SEG = 8                # recurrence segments
TS = T // SEG          # 64
W = 16                 # warmup steps
NSTEP = TS + W         # 80 wall steps per direction
NI = NB * SEG          # 64 chain instances (col order: e*SEG + s)
CH = 8                 # JIT pregate chunk (steps)
NCH = NSTEP // CH      # 10
IB = TS + 2 * W        # 96 i-slots per (e,s) block in xT
XC = NB * SEG * IB     # 6144 xT columns; col = (e*SEG+s)*IB + i; t = s*TS+i-W
KCH = [(0, 128, 128), (128, 256, 128), (256, 301, 45)]  # (row0, row1, kw); 300=mask/bias row

_CACHE = {}


def _build_program(debug=False):
    import concourse.bass as bass
    import concourse.tile as tile
    from concourse import bacc, mybir

    f32 = mybir.dt.float32
    f32r = mybir.dt.float32r
    bf16 = mybir.dt.bfloat16
    i32 = mybir.dt.int32
    AF = mybir.ActivationFunctionType
    OP = mybir.AluOpType
    AX = mybir.AxisListType

    nc = bacc.Bacc("TRN2", target_bir_lowering=False, debug=False,
                   enable_asserts=False, num_devices=NCORES)

    def inp(name, shape, dtype):
        return nc.dram_tensor(name, shape, dtype, kind="ExternalInput").ap()

    WORDS = inp("words", [128, 32], i32)            # masked ids; tok = c*128+p
    ETAB = inp("etab", [V + 1, E], f32r)            # + zero row V
    WIH = {d: inp(f"wihT_{d}", [384, G4], f32r) for d in "fb"}
    WHH = {d: inp(f"whhT_{d}", [H, G4], bf16) for d in "fb"}
    MASKXT = inp("maskxt", [1, XC], f32r)
    IDENT = inp("ident", [128, 128], f32r)
    WOUT = inp("woutT", [2 * H, L2], f32)
    BOUT = inp("bout", [L2, 1], f32)
    TSELF = [inp(f"tself{g}", [128, T * 32], f32) for g in range(2)]
    TSELB = [inp(f"tselb{g}", [128, T * 32], f32) for g in range(2)]
    MASKV = [inp(f"maskv{g}", [128, T], f32) for g in range(2)]
    TSTART = inp("tstart_col", [128, 1], f32)
    TSTOP = inp("tstop_col", [128, 1], f32)
    IOTAREV = inp("iotarev512", [128, T], f32)
    MASKSCR = [inp(f"maskscr{g}", [128, 16], f32) for g in range(2)]

    # scrambled tag layout: pathout[g, 32e+i, u] = tag(example 4g+e, t=32u+i)
    PATH_OUT = nc.dram_tensor("pathout", [2, 128, 16], i32,
                              kind="ExternalOutput").ap()
    if debug:
        DBG_H = {d: nc.dram_tensor(f"dbg_h_{d}", [128, 2, NB, T], f32,
                                   kind="ExternalOutput").ap() for d in "fb"}
        DBG_FV = [nc.dram_tensor(f"dbg_fv{g}", [128, T], f32,
                                 kind="ExternalOutput").ap() for g in range(2)]
        DBG_PF = [nc.dram_tensor(f"dbg_pf{g}", [128, T], f32,
                                 kind="ExternalOutput").ap() for g in range(2)]
        DBG_PB = [nc.dram_tensor(f"dbg_pb{g}", [128, T], f32,
                                 kind="ExternalOutput").ap() for g in range(2)]

    with tile.TileContext(nc) as tc:
        with ExitStack() as ctx:
            cst = ctx.enter_context(tc.tile_pool(name="cst", bufs=1))

            # ---- constants ----
            whh_sb = {d: cst.tile([128, 2, G4], bf16, tag=f"whh{d}", name=f"whh{d}")
                      for d in "fb"}
            for d in "fb":
                nc.sync.dma_start(whh_sb[d][:, 0, :], WHH[d][0:128, :])
                nc.sync.dma_start(whh_sb[d][:, 1, :], WHH[d][128:256, :])
            wih_sb = {d: cst.tile([128, 3, G4], f32r, tag=f"wih{d}", name=f"wih{d}")
                      for d in "fb"}
            for d in "fb":
                for k in range(3):
                    r0, r1, kw = KCH[k]
                    nc.sync.dma_start(wih_sb[d][:kw, k, :],
                                      WIH[d][k * 128:k * 128 + kw, :])
            if True:
                h_out = {d: cst.tile([128, 2, NB, T], f32, tag=f"ho{d}",
                                     name=f"ho{d}") for d in "fb"}

                # ================= phase 0: gather + transpose =================
                with tc.tile_pool(name="xtp", bufs=1) as xtp:
                    xt = [xtp.tile([128, XC], f32r, tag=f"xt{k}", name=f"xt{k}")
                          for k in range(3)]
                    # zero pads: (s=0, i<W) and (s=SEG-1, i>=IB-W) per (e)
                    for k in range(3):
                        kw = KCH[k][2]
                        kwz = kw if k < 2 else kw - 1     # mask row written below
                        z0 = xt[k][:kwz, :].bitcast(mybir.dt.uint32).rearrange(
                            "p (q i) -> p q i", i=IB)
                        nc.vector.memset(z0[:, 0::SEG, 0:W], 0)
                        nc.vector.memset(z0[:, SEG - 1::SEG, IB - W:IB], 0)
                    nc.sync.dma_start(xt[2][44:45, :], MASKXT[:])

                    idx_all = cst.tile([128, 32], i32, tag="idx", name="idx")
                    nc.sync.dma_start(idx_all[:], WORDS[:])
                    with tc.tile_pool(name="ph0", bufs=4) as p0, \
                         tc.tile_pool(name="ph0ps", bufs=4, space="PSUM") as p0ps:
                        ident = p0.tile([128, 128], f32r, tag="ident",
                                        name="ident")
                        nc.sync.dma_start(ident[:], IDENT[:])
                        for c in range(32):
                            e, p4 = divmod(c, 4)
                            s0 = 2 * p4          # chunk covers segments s0, s0+1

                            def xbase(s):
                                return (e * SEG + s) * IB

                            idxc = p0.tile([128, 1], i32, tag="idxc", name="idxc")
                            nc.vector.tensor_copy(idxc[:], idx_all[:, c:c + 1])
                            femb = p0.tile([128, E], f32r, tag="femb", name="femb")
                            nc.gpsimd.indirect_dma_start(
                                out=femb[:], out_offset=None, in_=ETAB[:],
                                in_offset=bass.IndirectOffsetOnAxis(
                                    ap=idxc[:, :1], axis=0))
                            for k in range(3):
                                r0, r1, kw = KCH[k]
                                kwe = min(r1, E) - r0          # 128/128/44 rows
                                pst = p0ps.tile([128, 128], f32r, tag="tps",
                                                name="tps")
                                nc.tensor.transpose(pst[:kwe, :],
                                                    femb[:, r0:r0 + kwe],
                                                    ident[:])
                                # (dst_col, src_col, width) per copy
                                cps = [(xbase(s0) + W, 0, TS),
                                       (xbase(s0 + 1) + W, TS, TS),
                                       (xbase(s0) + IB - W, TS, W),
                                       (xbase(s0 + 1), TS - W, W)]
                                if s0 + 2 < SEG:
                                    cps.append((xbase(s0 + 2), 128 - W, W))
                                if s0 >= 1:
                                    cps.append((xbase(s0 - 1) + IB - W, 0, W))
                                for j, (dc, sc, wd) in enumerate(cps):
                                    if (c + k + j) % 2:
                                        nc.scalar.copy(
                                            xt[k][:kwe, dc:dc + wd],
                                            pst[:kwe, sc:sc + wd])
                                    else:
                                        nc.vector.tensor_copy(
                                            xt[k][:kwe, dc:dc + wd],
                                            pst[:kwe, sc:sc + wd])

                    # ============ phase 2: fused JIT projection + recurrence ====
                    with tc.tile_pool(name="pgps", bufs=2, space="PSUM") as pgps, \
                         tc.tile_pool(name="recps", bufs=2, space="PSUM") as rps, \
                         tc.tile_pool(name="pgstg", bufs=1) as stgp, \
                         tc.tile_pool(name="chain", bufs=1) as chp, \
                         tc.tile_pool(name="state", bufs=1) as stp:
                        c_t = {d: stp.tile([128, 2, NI], f32, tag=f"c{d}",
                                           name=f"c{d}") for d in "fb"}
                        hbf = {d: stp.tile([128, 2, NI], bf16, tag=f"hb{d}",
                                           name=f"hb{d}") for d in "fb"}
                        for d in "fb":
                            nc.vector.memset(c_t[d][:], 0)
                            nc.vector.memset(hbf[d][:], 0)

                        HCH = CH // 2
                        pg_stage = {d: [None] * NCH for d in "fb"}

                        def issue_gemm_unit(kchunk, d, m):
                            """JIT pregates for (dir d, gate block m) of chunk."""
                            i0 = kchunk * CH
                            if d == "f":
                                sl = (i0, i0 + CH)
                            else:
                                sl = (IB - i0 - CH, IB - i0)
                            if pg_stage[d][kchunk] is None:
                                pg_stage[d][kchunk] = [
                                    stgp.tile([128, 8, HCH, NI], f32,
                                              tag=f"pg{h}{d}", name=f"pg{h}{d}")
                                    for h in range(2)]
                            ps = pgps.tile([128, CH, NI], f32,
                                           tag=f"pgm{d}", name=f"pgm{d}")
                            for k in range(3):
                                kw = KCH[k][2]
                                rhs = xt[k][:kw, :].rearrange(
                                    "p (q i) -> p q i", i=IB)[:, :, sl[0]:sl[1]
                                    ].rearrange("p q i -> p i q")
                                nc.tensor.matmul(
                                    ps[:], lhsT=wih_sb[d][:kw, k,
                                                          m * 128:(m + 1) * 128],
                                    rhs=rhs, start=(k == 0), stop=(k == 2))
                            for h in range(2):
                                nc.vector.tensor_copy(
                                    pg_stage[d][kchunk][h][:, m, :, :],
                                    ps[:, h * HCH:(h + 1) * HCH, :])

                        def step(d, i):
                            kchunk, ii = divmod(i, CH)
                            cc = ii if d == "f" else CH - 1 - ii
                            h, off = divmod(cc, HCH)
                            pg = pg_stage[d][kchunk][h][:, :, off, :]  # [128,8,NI]
                            ps = rps.tile([128, 8, NI], f32, tag=f"g{d}",
                                          name=f"g{d}")
                            nc.scalar.copy(ps[:], pg)
                            for m in range(8):
                                for kk in range(2):
                                    nc.tensor.matmul(
                                        ps[:, m, :],
                                        lhsT=whh_sb[d][:, kk,
                                                       m * 128:(m + 1) * 128],
                                        rhs=hbf[d][:, kk, :],
                                        start=False, stop=(kk == 1),
                                        skip_group_check=True)
                            # gate order (i, f, o, g); acts read PSUM directly
                            s_ifo = chp.tile([128, 6, NI], f32, tag=f"si{d}",
                                             name=f"si{d}")
                            nc.scalar.activation(s_ifo[:], ps[:, 0:6, :],
                                                 AF.Sigmoid)
                            t_g = chp.tile([128, 2, NI], f32, tag=f"tg{d}",
                                           name=f"tg{d}")
                            nc.scalar.activation(t_g[:], ps[:, 6:8, :], AF.Tanh)
                            tmp = chp.tile([128, 2, NI], f32, tag=f"tm{d}",
                                           name=f"tm{d}")
                            nc.vector.tensor_mul(tmp[:], s_ifo[:, 0:2, :], t_g[:])
                            nc.vector.tensor_mul(c_t[d][:], c_t[d][:],
                                                 s_ifo[:, 2:4, :])
                            nc.vector.tensor_add(c_t[d][:], c_t[d][:], tmp[:])
                            t_c = chp.tile([128, 2, NI], f32, tag=f"tc{d}",
                                           name=f"tc{d}")
                            nc.scalar.activation(t_c[:], c_t[d][:], AF.Tanh)
                            nc.vector.tensor_mul(hbf[d][:], t_c[:],
                                                 s_ifo[:, 4:6, :])
                            if i >= W:
                                r = i - W if d == "f" else TS - 1 - (i - W)
                                dst = h_out[d][:, :, :, :].rearrange(
                                    "p k e (s r) -> p k e s r", s=SEG)[:, :, :, :, r]
                                nc.gpsimd.tensor_tensor(
                                    dst, t_c[:].rearrange("p k (e s) -> p k e s",
                                                          s=SEG),
                                    s_ifo[:, 4:6, :].rearrange(
                                        "p k (e s) -> p k e s", s=SEG),
                                    op=OP.mult)

                        for d in "fb":
                            for m in range(8):
                                issue_gemm_unit(0, d, m)
                        for kchunk in range(NCH):
                            for ii in range(CH):
                                i = kchunk * CH + ii
                                if kchunk + 1 < NCH and ii < 8:
                                    u = 2 * ii
                                    for uu in (u, u + 1):
                                        d2, m2 = ("f", uu) if uu < 8 else \
                                                 ("b", uu - 8)
                                        issue_gemm_unit(kchunk + 1, d2, m2)
                                step("f", i)
                                step("b", i)

                # late constants (opened after xt frees its SBUF)
                late = ctx.enter_context(tc.tile_pool(name="late", bufs=1))
                maskv = [late.tile([128, T], f32, tag=f"mv{g}", name=f"mv{g}")
                         for g in range(2)]
                iotarev = late.tile([128, T], f32, tag="iota", name="iota")
                tstart = late.tile([128, 1], f32, tag="tstart", name="tstart")
                tstop = late.tile([128, 1], f32, tag="tstop", name="tstop")
                maskscr = [late.tile([128, 16], f32, tag=f"ms{g}", name=f"ms{g}")
                           for g in range(2)]
                for apdst, apsrc in ((maskv[0], MASKV[0]), (maskv[1], MASKV[1]),
                                     (iotarev, IOTAREV), (tstart, TSTART),
                                     (tstop, TSTOP), (maskscr[0], MASKSCR[0]),
                                     (maskscr[1], MASKSCR[1])):
                    nc.sync.dma_start(apdst[:], apsrc[:])
                featsvit = [late.tile([128, T], f32, tag=f"fv{g}", name=f"fv{g}")
                            for g in range(2)]

                # ================= phase 3: feats =================
                if debug:
                    for d in "fb":
                        nc.sync.dma_start(DBG_H[d][:], h_out[d][:])
                with tc.tile_pool(name="fw", bufs=1) as fwp, \
                     tc.tile_pool(name="fps", bufs=4, space="PSUM") as fps:
                    wout = fwp.tile([128, 4, L2], f32, tag="wo", name="wo")
                    for k in range(4):
                        nc.sync.dma_start(wout[:, k, :],
                                          WOUT[k * 128:(k + 1) * 128, :])
                    bout_sb = fwp.tile([L2, 1], f32, tag="bo", name="bo")
                    nc.sync.dma_start(bout_sb[:], BOUT[:])
                    feats_sb = fwp.tile([L2, NB * T], f32, tag="fs", name="fs")
                    hsrc = [h_out["f"][:, 0, :, :], h_out["f"][:, 1, :, :],
                            h_out["b"][:, 0, :, :], h_out["b"][:, 1, :, :]]
                    for n in range(8):
                        ps = fps.tile([L2, 512], f32, tag="fp", name="fp")
                        for k in range(4):
                            flat = hsrc[k].rearrange("p e t -> p (e t)")
                            nc.tensor.matmul(
                                ps[:], lhsT=wout[:, k, :],
                                rhs=flat[:, n * 512:(n + 1) * 512],
                                start=(k == 0), stop=(k == 3))
                        nc.scalar.activation(feats_sb[:, n * 512:(n + 1) * 512],
                                             ps[:], AF.Identity,
                                             bias=bout_sb[:, 0:1])
                    for g in range(2):
                        nc.vector.memset(featsvit[g][:], 0)
                        for e in range(4):
                            b = 4 * g + e
                            nc.sync.dma_start(
                                featsvit[g][32 * e:32 * e + L2, :],
                                feats_sb[:, b * T:(b + 1) * T])
                        nc.vector.tensor_mul(featsvit[g][:], featsvit[g][:],
                                             maskv[g][:])

            # ================= phase 4: dual viterbi scans =================
            if debug:
                for g in range(2):
                    nc.sync.dma_start(DBG_FV[g][:], featsvit[g][:])
            pa_f = [late.tile([128, T], f32, tag=f"paf{g}", name=f"paf{g}")
                    for g in range(2)]
            pa_b = [late.tile([128, T], f32, tag=f"pab{g}", name=f"pab{g}")
                    for g in range(2)]
            CHV = 16
            with tc.tile_pool(name="vstr", bufs=2) as vsp:
                pff = [late.tile([128, 1], f32, tag=f"pff{g}", name=f"pff{g}")
                       for g in range(2)]
                pfb = [late.tile([128, 1], f32, tag=f"pfb{g}", name=f"pfb{g}")
                       for g in range(2)]
                for g in range(2):
                    nc.vector.tensor_copy(pa_f[g][:, 0:1], tstart[:])
                    nc.vector.tensor_copy(pa_b[g][:, T - 1:T], tstop[:])
                    nc.scalar.activation(pff[g][:], tstart[:], AF.Identity,
                                         bias=featsvit[g][:, 0:1])
                    nc.scalar.activation(pfb[g][:], tstop[:], AF.Identity,
                                         bias=featsvit[g][:, T - 1:T])
                strm = {}
                NQC = T // CHV

                def load_chunk(kind, g, qc):
                    src = (TSELF if kind == "f" else TSELB)[g]
                    tl = vsp.tile([128, CHV * 32], f32, tag=f"st{kind}{g}",
                                  name=f"st{kind}{g}")
                    nc.sync.dma_start(
                        tl[:], src[:, qc * CHV * 32:(qc + 1) * CHV * 32])
                    strm[(kind, g, qc % 2)] = tl

                for g in range(2):
                    for kind in "fb":
                        load_chunk(kind, g, 0)
                        load_chunk(kind, g, 1)
                for q in range(1, T):
                    tau = T - 1 - q
                    for g in range(2):
                        qc = q // CHV
                        if q % CHV == 0 and qc + 1 < NQC:
                            load_chunk("f", g, qc + 1)
                            load_chunk("b", g, qc + 1)
                        qq = q % CHV
                        slot_f = strm[("f", g, qc % 2)][:, qq * 32:(qq + 1) * 32]
                        cur = vsp.tile([128, 32], f32, tag=f"cf{g}",
                                       name=f"cf{g}")
                        nc.scalar.activation(cur[:], slot_f, AF.Identity,
                                             bias=pff[g][:, 0:1])
                        nc.vector.tensor_reduce(
                            pa_f[g][:, q:q + 1], cur[:], axis=AX.X, op=OP.max,
                            apply_transpose=True)
                        slot_b = strm[("b", g, qc % 2)][:, qq * 32:(qq + 1) * 32]
                        curb = vsp.tile([128, 32], f32, tag=f"cb{g}",
                                        name=f"cb{g}")
                        nc.scalar.activation(curb[:], slot_b, AF.Identity,
                                             bias=pfb[g][:, 0:1])
                        nc.vector.tensor_reduce(
                            pa_b[g][:, tau:tau + 1], curb[:], axis=AX.X,
                            op=OP.max, apply_transpose=True)
                        if q < T - 1:
                            if g == 0:
                                nc.scalar.activation(
                                    pff[g][:], pa_f[g][:, q:q + 1], AF.Identity,
                                    bias=featsvit[g][:, q:q + 1])
                                nc.scalar.activation(
                                    pfb[g][:], pa_b[g][:, tau:tau + 1],
                                    AF.Identity,
                                    bias=featsvit[g][:, tau:tau + 1])
                            else:
                                nc.vector.tensor_scalar(
                                    pff[g][:], pa_f[g][:, q:q + 1],
                                    featsvit[g][:, q:q + 1], None, op0=OP.add)
                                nc.vector.tensor_scalar(
                                    pfb[g][:], pa_b[g][:, tau:tau + 1],
                                    featsvit[g][:, tau:tau + 1], None,
                                    op0=OP.add)

            # ================= phase 5: bulk max-marginal decode ============
            if debug:
                for g in range(2):
                    nc.sync.dma_start(DBG_PF[g][:], pa_f[g][:])
                    nc.sync.dma_start(DBG_PB[g][:], pa_b[g][:])
            with tc.tile_pool(name="dec", bufs=1) as dp:
                for g in range(2):
                    s_all = dp.tile([128, T], f32, tag=f"sa{g}", name=f"sa{g}")
                    nc.vector.tensor_add(s_all[:], pa_f[g][:], pa_b[g][:])
                    nc.vector.tensor_add(s_all[:], s_all[:], featsvit[g][:])
                    st = dp.tile([128, T], f32, tag=f"stt{g}", name=f"stt{g}")
                    nc.vector.transpose(st[:], s_all[:])
                    stv = st[:].rearrange("p (u v) -> p u v", v=32)
                    maxv = dp.tile([128, 16], f32, tag=f"mx{g}", name=f"mx{g}")
                    nc.vector.tensor_reduce(maxv[:], stv, axis=AX.X, op=OP.max)
                    eq = dp.tile([128, T], f32, tag=f"eq{g}", name=f"eq{g}")
                    mvb = maxv[:].rearrange("p u -> p u ()").broadcast_to(
                        (128, 16, 32))
                    nc.vector.tensor_tensor(
                        eq[:].rearrange("p (u v) -> p u v", v=32), stv, mvb,
                        op=OP.is_equal)
                    nc.vector.tensor_mul(eq[:], eq[:], iotarev[:])
                    brv = dp.tile([128, 16], f32, tag=f"br{g}", name=f"br{g}")
                    nc.vector.tensor_reduce(
                        brv[:], eq[:].rearrange("p (u v) -> p u v", v=32),
                        axis=AX.X, op=OP.max)
                    nc.vector.tensor_scalar(brv[:], brv[:], -1.0, 31.0,
                                            op0=OP.mult, op1=OP.add)
                    nc.vector.tensor_mul(brv[:], brv[:], maskscr[g][:])
                    tagi = dp.tile([128, 16], i32, tag=f"ti{g}", name=f"ti{g}")
                    nc.vector.tensor_copy(tagi[:], brv[:])
                    nc.sync.dma_start(PATH_OUT[g, :, :], tagi[:])

    nc.compile()
    return nc


def _prep_inputs(inputs):
    f32 = np.float32
    bf = ml_dtypes.bfloat16

    mask = np.ascontiguousarray(np.asarray(inputs["mask"])).astype(f32)
    words = np.ascontiguousarray(np.asarray(inputs["batch_word"])).astype(np.int32)
    words_m = np.where(mask > 0, words, V).astype(np.int32)

    etab = np.zeros((V + 1, E), f32)
    etab[:V] = np.asarray(inputs["emb_table"], f32)
    trans = np.asarray(inputs["trans"], f32)

    # gate order permutation (i, f, g, o) -> (i, f, o, g); blocks of 128 cols
    blk = [0, 1, 2, 3, 6, 7, 4, 5]

    def permute_cols(w):            # w: [4H, x] rows are gate blocks
        return np.concatenate([w[128 * b:128 * (b + 1)] for b in blk], axis=0)

    shared = {"etab": etab, "ident": np.eye(128, dtype=f32)}
    for d, (wih, whh, bih, bhh) in (
            ("f", ("Wih_f", "Whh_f", "bih_f", "bhh_f")),
            ("b", ("Wih_b", "Whh_b", "bih_b", "bhh_b"))):
        Wih = permute_cols(np.asarray(inputs[wih], f32))       # [4H, E]
        Whh = permute_cols(np.asarray(inputs[whh], f32))       # [4H, H]
        bias = permute_cols((np.asarray(inputs[bih], f32)
                             + np.asarray(inputs[bhh], f32))[:, None])[:, 0]
        wt = np.zeros((384, G4), f32)
        wt[0:128] = Wih.T[0:128]
        wt[128:256] = Wih.T[128:256]
        wt[256:300] = Wih.T[256:300]
        wt[300] = bias
        shared[f"wihT_{d}"] = wt
        shared[f"whhT_{d}"] = np.ascontiguousarray(Whh.T).astype(bf)  # [H, 4H]

    shared["woutT"] = np.ascontiguousarray(
        np.asarray(inputs["W_out"], f32).T)                    # [2H, L2]
    shared["bout"] = np.ascontiguousarray(
        np.asarray(inputs["b_out"], f32).reshape(L2, 1))

    id_m = np.where(np.eye(L2, dtype=bool), 0.0, NEG).astype(f32)
    tr22 = trans[:L2, :L2]
    # 32x32 padded variants; pads PAD
    trans32 = np.full((32, 32), PAD, f32); trans32[:L2, :L2] = tr22
    id32 = np.full((32, 32), PAD, f32); id32[:L2, :L2] = id_m

    tstart_col = np.full((128, 1), PAD, f32)
    tstop_col = np.full((128, 1), PAD, f32)
    for e in range(4):
        tstart_col[32 * e:32 * e + L2, 0] = trans[START, :L2]
        tstop_col[32 * e:32 * e + L2, 0] = trans[:L2, STOP]
    iotarev512 = np.zeros((128, T), f32)
    for u in range(16):
        iotarev512[:, 32 * u:32 * u + 32] = (31.0 - np.arange(32, dtype=f32))[None, :]
    shared["tstart_col"] = tstart_col
    shared["tstop_col"] = tstop_col
    shared["iotarev512"] = iotarev512

    in_maps = []
    for c in range(NCORES):
        m = dict(shared)
        bsl = slice(c * NB, (c + 1) * NB)
        wl = words_m[bsl].reshape(NB * T)
        m["words"] = np.ascontiguousarray(wl.reshape(32, 128).T)
        mk = mask[bsl]                                        # [8, T]
        # maskxt row: col (e*SEG+s)*IB + i -> t = s*TS + i - W
        mrow = np.zeros((1, XC), f32)
        for e in range(NB):
            for s in range(SEG):
                base = (e * SEG + s) * IB
                t0 = s * TS - W
                for i in range(IB):
                    t = t0 + i
                    if 0 <= t < T:
                        mrow[0, base + i] = mk[e, t]
        m["maskxt"] = mrow
        for g in range(2):
            mv = np.zeros((128, T), f32)
            msc = np.zeros((128, 16), f32)
            tsf = np.full((128, T * 32), PAD, f32)
            tsb = np.full((128, T * 32), PAD, f32)
            for e in range(4):
                ex = 4 * g + e
                mv[32 * e:32 * e + L2, :] = mk[ex][None, :]
                for i in range(32):
                    msc[32 * e + i, :] = mk[ex, i::32]
                # fwd slots q=1..T-1: tsel[q] = m[q] ? trans : id ([j, i])
                me = mk[ex]                                   # [T]
                sel = np.where(me[:, None, None] > 0, trans32[None],
                               id32[None])                    # [T, 32, 32]
                tsf[32 * e:32 * e + 32, :] = np.ascontiguousarray(
                    sel.transpose(1, 0, 2).reshape(32, T * 32))
                # bwd slot q (q=1..T-1): tau = T-1-q uses tsel[tau+1] transposed
                selb = np.where(me[None, :, None] > 0, trans32.T[:, None, :],
                                id32[:, None, :])             # [32, T, 32] (i, t, j)
                # slot q content cols q*32+j = sel at t=tau+1=T-q
                ordb = selb[:, (T - np.arange(T)) % T, :]     # q=0 unused
                tsb[32 * e:32 * e + 32, :] = np.ascontiguousarray(
                    ordb.reshape(32, T * 32))
            m[f"maskv{g}"] = mv
            m[f"maskscr{g}"] = msc
            m[f"tself{g}"] = tsf
            m[f"tselb{g}"] = tsb
        in_maps.append(m)
    return in_maps


def kernel(**inputs):
    from concourse.bass_utils import run_bass_kernel_spmd

    debug = bool(int(os.environ.get("KDEBUG", "0")))
    key = ("nc", debug)
    if key not in _CACHE:
        _CACHE[key] = _build_program(debug)
    nc = _CACHE[key]

    in_maps = _prep_inputs(inputs)
    res = run_bass_kernel_spmd(nc, in_maps, core_ids=list(range(NCORES)))
    _CACHE["last_results"] = res
    out = np.zeros((B, T), np.int32)
    for c in range(NCORES):
        scr = res.results[c]["pathout"]          # [2, 128, 16]
        for g in range(2):
            blk = scr[g].reshape(4, 32, 16)      # [e, i, u]
            out[c * NB + 4 * g:c * NB + 4 * g + 4] = (
                blk.transpose(0, 2, 1).reshape(4, T))
    return out.astype(np.int32)
